# revision 29
# baseline (speedup 1.0000x reference)
"""Dynamic conv2d (CondConv-style) Trainium2 Bass kernel, v3.

Problem: per-sample routing (GAP -> FC -> sigmoid over K=8 experts), expert
weight aggregation, then a per-sample 3x3 conv (pad=1) plus aggregated bias.

Sharding: data-parallel over batch across 8 NeuronCores (4 samples/core);
the K-expert weight bank is replicated to every core.

v3: 111.5us vs the 131.7us v2 baseline (TimelineSim cost-model estimate,
which is what the harness reports).  Changes vs v2:
  - Routing GAP is computed ON THE PE as 27 tiny accumulating matmuls
    (x-chunk^T @ fcw); a ones-matmul then reduces partitions AND broadcasts
    the logits to all 128 partitions in one shot.  This removes the big
    GAP reductions from ACT/DVE entirely and costs ~100ns of PE time.
  - The expert bank is stored g-block-major ([c, m, g, k, co]) so sample
    0's aggregation can stream group-by-group behind the bank DMA, letting
    the first conv matmul issue at ~8us instead of ~18us.
  - Aggregation uses tensor_scalar_mul products (4x DVE mode) + a
    tensor_tensor add tree (2x mode), optionally splitting products to ACT.
  - Warmup matmuls on a zeros tile keep the PE busy (and the p-state ramp
    warm) from ~1us until the first conv matmul; the cost model halves
    matmul speed for 3us after any PE idle gap.
  - Small constant inputs (fcw/fcb/bias) ride the SWDGE (gpsimd) DMA path
    so they don't consume HWDGE pipeline slots ahead of x0.
  - Sample 0 conv runs (m, g, n): 7 PSUM banks accumulate one tap pass at
    a time so only one aggregated g-block is needed to start.  Samples 1-3
    run (m, n, g) so evictions spread evenly.

NOTE (measured on device, v2): a strided conv rhs AP (packed cols, no pad
columns) simulates faster but returns WRONG results on hardware -- the PE
ifmap must be a flat contiguous slice.  Don't retry without verifying
numerics.
"""

import numpy as np

B, C_IN, H, W = 32, 128, 56, 56
C_OUT, KS, K = 256, 3, 8
N_CORES = 8
B_LOC = B // N_CORES  # 4 samples per core

WP = W + 2                 # padded row width: 58
NPAD = (H + 2) * WP        # padded spatial size: 3364
GUARD = 8                  # front guard cols so every tap shift is in bounds
XBUF = 3456                # 27*128: front guard 8 + padded image + back pad
IN0 = GUARD + WP + 1       # xbuf col of output pixel (0,0)'s center tap
NG = KS * KS               # 9 tap blocks
BLK = K * 128              # 1024 cols per (m, g) bank block
TAP_COLS = NG * 128        # 1152 columns of aggregated weights per m-half
HW = H * W                 # 3136
M_TILES = C_OUT // 128     # 2
ROWS_PER_CHUNK = 8
N_ROW_CHUNKS = H // ROWS_PER_CHUNK  # 7
OW = ROWS_PER_CHUNK * W             # 448 output cols per chunk
CW = ROWS_PER_CHUNK * WP            # 464 psum cols per chunk
N_GAP_CHUNKS = XBUF // 128          # 27

_CACHE = {}


def _make_tile_context_cls():
    import concourse.mybir as mybir
    from concourse.tile import TileContext
    from concourse.vector_clock import ScopedClock

    class SplitDrainTileContext(TileContext):
        """Walrus in this container caps sync waits per CTRL instruction;
        the Tile tail drain can accumulate more. Keep one wait on the drain
        and move the rest onto dedicated nops."""

        def _drain_and_barrier(self, tick_clock, wait_clock):
            drain_inst = self.nc.sync.drain()
            wait_clock.add_sem_waits(
                drain_inst.ins, ScopedClock({None: tick_clock.global_clock})
            )
            si = drain_inst.ins.sync_info
            if si is not None and len(si.on_wait) > 1:
                waits = list(si.on_wait)
                drain_inst.ins.sync_info = mybir.SyncInfo(
                    on_wait=waits[:1], on_update=list(si.on_update)
                )
                for w in waits[1:]:
                    n = self.nc.sync.nop(nofuse=True)
                    n.ins.sync_info = mybir.SyncInfo(on_wait=[w], on_update=[])
            self.nc.all_engine_barrier()
            assert self.sems is not None
            popped = self.nc._tile_sem_poison_stack.pop()
            assert popped is self._sem_poison
            self.nc.clear_and_free_semaphores(list(self.sems.allocated().values()))
            self.nc.all_engine_barrier()

    return SplitDrainTileContext


def _split_excess_waits(nc, cap=1):
    """The walrus build in this container rejects instructions carrying more
    than ~1-2 sem waits (setupSyncWait: 'Too many sync wait commands').
    Conservatively keep at most `cap` waits per instruction and move the rest
    onto same-engine NoOps inserted immediately before it."""
    import concourse.mybir as mybir

    for f in nc.m.functions:
        for blk in f.blocks:
            insts = blk.instructions
            if not any(
                i.sync_info is not None and len(i.sync_info.on_wait) > cap
                for i in insts
            ):
                continue
            new_insts = []
            for inst in insts:
                si = inst.sync_info
                if si is not None and len(si.on_wait) > cap:
                    waits = list(si.on_wait)
                    for j, w in enumerate(waits[cap:]):
                        noop = mybir.InstNoOp(
                            name=f"{inst.name}-waitsplit{j}",
                            engine=inst.engine,
                            ins=[],
                            outs=[],
                            bass_nofuse=True,
                            sync_info=mybir.SyncInfo(on_wait=[w], on_update=[]),
                        )
                        nc.register_instruction(noop)
                        new_insts.append(noop)
                    inst.sync_info = mybir.SyncInfo(
                        on_wait=waits[:cap], on_update=list(si.on_update)
                    )
                new_insts.append(inst)
            blk.instructions = new_insts


# -- schedule tunables (tuned against the TimelineSim cost model) -----------
TUNE = dict(
    warm_a=7,    # warmups before the x0-half-1 gap-chunk matmuls
    warm_b=11,   # warmups between half-1 and half-2 chunk matmuls
    warm_c=1,    # warmups before the ones-reduce matmul
    warm_d=10,   # warmups before conv g0 (cover route tail + block-0 agg)
    warm_ap=464,
    g_split=(),      # expert-pair DMA splits for the first 1-block groups
    tail_split=False,  # split the final chunk into two 4-row pieces
    # sample-0 aggregation group sizes (g-blocks per group), per m-half.
    # m1 groups must match the m1 bank DMA chunking below.
    s0_groups_m0=(1, 1, 1, 2, 2, 2),
    s0_groups_m1=(3, 3, 3),
    s0_prods_small="DDDAAAPP",   # product engines, 1-block groups
    s0_small_list=(),            # optional per-small-block prods override
    s0_prods_big="DDDDAAPP",     # product engines, multi-block groups
    s0_pool_pairs_small=(6,),    # first-level pair-adds moved to Pool
    s0_pool_pairs_big=(6,),
    # filler warmups inserted before (m, g) tap passes of sample-0's conv
    s0_fill={},
    mid1_gap=(0, 7),  # conv-0 tap pass hosting sample-1 gap matmuls
    mid1_m0=(1, 0),
    mid1_m1=(1, 4),
    # product engines for steady-state samples
    prods="DDDDPPAA",
    pool_pairs=(4,),
)


def _build_bass():
    import concourse.bass as bass
    import concourse.mybir as mybir

    F32 = mybir.dt.float32
    F16 = mybir.dt.float16
    SIG = mybir.ActivationFunctionType.Sigmoid
    IDENT = mybir.ActivationFunctionType.Identity
    COPY = mybir.ActivationFunctionType.Copy
    MULT = mybir.AluOpType.mult
    ADD = mybir.AluOpType.add

    SplitDrainTileContext = _make_tile_context_cls()

    nc = bass.Bass()
    xs = nc.dram_tensor("xs", [B_LOC, C_IN, XBUF], F16, kind="ExternalInput")
    # g-block-major bank: col = ((m*9 + g)*8 + k)*128 + co
    wT = nc.dram_tensor("wT", [C_IN, M_TILES * NG * BLK], F16,
                        kind="ExternalInput")
    fcwT = nc.dram_tensor("fcwT", [C_IN, K], F16, kind="ExternalInput")
    # fcb broadcast, pre-scaled by HW/128 (see routing notes below)
    fcbb = nc.dram_tensor("fcbb", [C_IN, K], F32, kind="ExternalInput")
    biasT = nc.dram_tensor("biasT", [C_OUT, K], F32, kind="ExternalInput")
    out = nc.dram_tensor("out", [B_LOC, C_OUT, H * W], F16,
                         kind="ExternalOutput")

    # tap g = kh*3+kw reads the padded image shifted by (kh-1, kw-1)
    TAP_DELTA = [(kh - 1) * WP + (kw - 1) for kh in range(KS) for kw in range(KS)]
    inv_hw = 1.0 / float(HW)

    with SplitDrainTileContext(nc) as tc:
        with (
            tc.tile_pool(name="const", bufs=1) as constp,
            tc.tile_pool(name="agg", bufs=2) as aggp,
            tc.tile_pool(name="small", bufs=6) as smallp,
            tc.tile_pool(name="osb", bufs=8) as outp,
            tc.tile_pool(name="rps", bufs=1, space="PSUM") as rpsp,
            tc.tile_pool(name="cps", bufs=7, space="PSUM") as cpsp,
        ):
            # --- persistent tiles -------------------------------------
            bank = constp.tile([128, M_TILES * NG * BLK], F16, name="bank",
                               tag="bank")
            fcwT_sb = constp.tile([C_IN, K], F16, name="fcwT_sb", tag="fcwT")
            fcbb_sb = constp.tile([C_IN, K], F32, name="fcbb_sb", tag="fcbb")
            biasT_sb = [
                constp.tile([128, K], F32, name=f"biasT{m}", tag=f"biasT{m}")
                for m in range(M_TILES)
            ]
            ones128 = constp.tile([128, 128], F16, name="ones128", tag="ones")
            z16 = constp.tile([128, TUNE["warm_ap"]], F16, name="z16", tag="z16")
            xbufs = [
                constp.tile([128, XBUF], F16, name=f"xbuf{b}", tag=f"xbuf{b}")
                for b in range(B_LOC)
            ]
            tmps = [
                constp.tile([128, TAP_COLS], F16, name=f"tmp{k}", tag=f"tmp{k}")
                for k in range(K)
            ]

            # --- input DMA schedule ---------------------------------------
            # fcw/fcbb ride SWDGE (gpsimd) FIRST: their descriptor gen runs
            # on the idle Pool engine and their tiny transfers grab the bus
            # before x0's big HWDGE transfers queue up.
            nc.gpsimd.dma_start(out=fcwT_sb[:, :], in_=fcwT[:, :])
            nc.gpsimd.dma_start(out=fcbb_sb[:, :], in_=fcbb[:, :])
            # z16 on DVE so the first warmup matmul isn't gated on Pool
            nc.vector.memset(z16[:, :], 0.0)
            nc.gpsimd.memset(ones128[:, :], 1.0)

            # x0 + bank + later x's on SP/HWDGE (bus order == this order)
            XH = XBUF // 2
            nc.sync.dma_start(out=xbufs[0][:, 0:XH], in_=xs[0][:, 0:XH])
            nc.sync.dma_start(out=xbufs[0][:, XH:XBUF], in_=xs[0][:, XH:XBUF])

            def load_bank(m, g0, ng):
                base = (m * NG + g0) * BLK
                nc.sync.dma_start(
                    out=bank[:, base:base + ng * BLK],
                    in_=wT[:, base:base + ng * BLK],
                )

            def load_x(b):
                nc.sync.dma_start(out=xbufs[b][:, :], in_=xs[b])

            def load_bank_experts(m, g, pairs):
                # split one g-block's DMA into expert-pair pieces so the
                # first products can start as soon as their experts land
                base = (m * NG + g) * BLK
                step = BLK // pairs
                for e in range(pairs):
                    nc.sync.dma_start(
                        out=bank[:, base + e * step:base + (e + 1) * step],
                        in_=wT[:, base + e * step:base + (e + 1) * step],
                    )

            # m0: fine-grained first blocks matching s0_groups_m0
            g0 = 0
            for i, ng in enumerate(TUNE["s0_groups_m0"]):
                if ng == 1 and i < len(TUNE["g_split"]):
                    load_bank_experts(0, g0, TUNE["g_split"][i])
                else:
                    load_bank(0, g0, ng)
                g0 += ng
            for m in range(M_TILES):
                nc.sync.dma_start(
                    out=biasT_sb[m][:, :], in_=biasT[m * 128:(m + 1) * 128, :]
                )
            load_x(1)
            g0 = 0
            for ng in TUNE["s0_groups_m1"]:
                load_bank(1, g0, ng)
                g0 += ng
            load_x(2)
            load_x(3)

            # --- PE warmup ------------------------------------------------
            WAP = TUNE["warm_ap"]

            def warm(n):
                for _ in range(n):
                    wps = cpsp.tile([128, CW], F32, name="wps", tag="ps")
                    nc.tensor.matmul(wps[:, 0:WAP], lhsT=z16[:, 0:128],
                                     rhs=z16[:, 0:WAP], start=True, stop=True)

            # --- routing --------------------------------------------------
            # GAP on the PE: rp[p,k] = sum_c fcw[k,c] * sum_i x[c, 128i+p];
            # S = rp + fcb*HW/128 (DVE); a ones-matmul replicates
            # sum_p S[p,k] to every partition; sigmoid(scale=1/HW) on ACT.
            def gap_mms(b, rp, chunks):
                for ci in chunks:
                    nc.tensor.matmul(
                        rp[:, 0:K],
                        lhsT=xbufs[b][:, ci * 128:(ci + 1) * 128],
                        rhs=fcwT_sb[:, 0:K],
                        start=(ci == 0), stop=(ci == N_GAP_CHUNKS - 1),
                    )

            def route_head(b):
                return rpsp.tile([128, K], F32, name=f"rp{b}", tag="rps")

            def route_mid(b, rp):
                S = smallp.tile([128, K], F16, name=f"S{b}", tag="S")
                nc.vector.tensor_tensor(out=S[:, 0:K], in0=rp[:, 0:K],
                                        in1=fcbb_sb[:, 0:K], op=ADD)
                return S

            def route_tail(b, S):
                ps2 = rpsp.tile([128, K], F32, name=f"ps2{b}", tag="rps")
                nc.tensor.matmul(ps2[:, 0:K], lhsT=ones128[:, 0:128],
                                 rhs=S[:, 0:K], start=True, stop=True)
                attn_bc = smallp.tile([128, K], F32, name=f"attn{b}", tag="attn")
                nc.scalar.activation(attn_bc[:, 0:K], ps2[:, 0:K], SIG,
                                     scale=inv_hw)
                return attn_bc

            def agg_bias(b, attn_bc):
                # aggregated bias: per-partition dot <biasT[co,:], attn>
                aggb = smallp.tile([128, M_TILES], F32, name=f"aggb{b}",
                                   tag="aggb")
                ttr = smallp.tile([128, K], F32, name=f"ttr{b}", tag="ttr")
                for m in range(M_TILES):
                    nc.vector.tensor_tensor(
                        out=ttr[:, 0:K], in0=biasT_sb[m][:, 0:K],
                        in1=attn_bc[:, 0:K], op=MULT,
                    )
                    nc.vector.reduce_sum(
                        aggb[:, m:m + 1], ttr[:, 0:K], axis=mybir.AxisListType.X,
                    )
                return aggb

            # --- aggregation ---------------------------------------------
            def bank_kview(m, g0, ng, k):
                """Strided AP: expert k's g-blocks [g0, g0+ng) of half m."""
                v = bank[:, (m * NG + g0) * BLK:(m * NG + g0 + ng) * BLK]
                return v.rearrange("p (g k c) -> p g k c", g=ng, k=K)[:, :, k, :]

            def tmp_flat(t, g0, ng):
                # tmps are per-half scratch: no m offset
                return t[:, g0 * 128:(g0 + ng) * 128]

            def tmp_view(t, g0, ng):
                return tmp_flat(t, g0, ng).rearrange("p (g c) -> p g c", g=ng)

            def agg_flat(t, m, g0, ng):
                return t[:, m * TAP_COLS + g0 * 128:
                         m * TAP_COLS + (g0 + ng) * 128]

            def aggregate_group(b, attn_bc, aggT, m, g0, ng, prods="DDDDDDDD",
                                pool_adds=()):
                """Aggregate g-blocks [g0, g0+ng) of half m into aggT.

                `prods[k]` assigns expert k's product to D(VE, 4x mode),
                A(CT scaled copy) or P(ool); the tensor_tensor add tree runs
                on DVE except the pair-adds whose left index is in `pool_adds` on Pool.
                aggT cols of the group are final after the last add.
                """
                for k in range(K):
                    dst = tmp_view(tmps[k], g0, ng)
                    src = bank_kview(m, g0, ng, k)
                    if prods[k] == "D":
                        nc.vector.tensor_scalar_mul(dst, src, attn_bc[:, k:k + 1])
                    elif prods[k] == "A":
                        nc.scalar.activation(dst, src, COPY,
                                             scale=attn_bc[:, k:k + 1])
                    else:
                        nc.gpsimd.tensor_scalar_mul(dst, src, attn_bc[:, k:k + 1])
                # first-level pair adds: (0,1)->0, (2,3)->2, (4,5)->4, (6,7)->6
                for k in (0, 2, 4, 6):
                    eng = nc.gpsimd if k in pool_adds else nc.vector
                    eng.tensor_tensor(
                        out=tmp_flat(tmps[k], g0, ng),
                        in0=tmp_flat(tmps[k], g0, ng),
                        in1=tmp_flat(tmps[k + 1], g0, ng), op=ADD,
                    )
                for k in (0, 4):
                    nc.vector.tensor_tensor(
                        out=tmp_flat(tmps[k], g0, ng),
                        in0=tmp_flat(tmps[k], g0, ng),
                        in1=tmp_flat(tmps[k + 2], g0, ng), op=ADD,
                    )
                nc.vector.tensor_tensor(
                    out=agg_flat(aggT, m, g0, ng),
                    in0=tmp_flat(tmps[0], g0, ng),
                    in1=tmp_flat(tmps[4], g0, ng), op=ADD,
                )

            def new_aggT(b):
                return aggp.tile([128, M_TILES * TAP_COLS], F16,
                                 name=f"aggT{b}", tag="aggT")

            def aggregate_half(b, attn_bc, aggT, m):
                aggregate_group(b, attn_bc, aggT, m, 0, NG,
                                prods=TUNE["prods"],
                                pool_adds=TUNE["pool_pairs"])

            # --- conv -----------------------------------------------------
            def evict(b, m, n, ps, aggb, on_act):
                osb = outp.tile([128, OW], F16, name=f"osb{b}_{m}_{n}",
                                tag="osb")
                ps_in = ps[:, 0:CW].rearrange("p (y w) -> p y w", w=WP)[:, :, 1:W + 1]
                osb_out = osb[:, 0:OW].rearrange("p (y w) -> p y w", w=W)
                if on_act:
                    nc.scalar.activation(osb_out, ps_in, IDENT,
                                         bias=aggb[:, m:m + 1], scale=1.0)
                else:
                    nc.vector.tensor_scalar_add(osb_out, ps_in, aggb[:, m:m + 1])
                nc.sync.dma_start(
                    out=out[b, m * 128:(m + 1) * 128, n * OW:(n + 1) * OW],
                    in_=osb[:, 0:OW],
                )

            def warm_fill(n):
                # conv-interleaved warmups: the 7 cps slots are all held by
                # accumulating conv tiles, so borrow the routing pool's slot
                for _ in range(n):
                    wps = rpsp.tile([128, CW], F32, name="wfill", tag="rps")
                    nc.tensor.matmul(wps[:, 0:WAP], lhsT=z16[:, 0:128],
                                     rhs=z16[:, 0:WAP], start=True, stop=True)

            def conv_mgn(b, aggT, get_aggb, mids=None):
                """(m, g, n): 7 PSUM banks accumulate tap-by-tap (sample 0)."""
                mids = mids or {}
                aggb = None
                for m in range(M_TILES):
                    pss = [cpsp.tile([128, CW], F32, name=f"ps{b}_{m}_{n}",
                                     tag="ps") for n in range(N_ROW_CHUNKS)]
                    for g in range(NG):
                        if (m, g) in mids:
                            mids[(m, g)]()
                        if (m, g) in TUNE["s0_fill"]:
                            warm_fill(TUNE["s0_fill"][(m, g)])
                        lhs = agg_flat(aggT, m, g, 1)
                        for n in range(N_ROW_CHUNKS):
                            p0 = (ROWS_PER_CHUNK * n + 1) * WP
                            base = GUARD + p0 + TAP_DELTA[g]
                            nc.tensor.matmul(
                                pss[n][:, 0:CW], lhsT=lhs,
                                rhs=xbufs[b][:, base:base + CW],
                                start=(g == 0), stop=(g == NG - 1),
                            )
                    if aggb is None:
                        aggb = get_aggb()
                    for n in range(N_ROW_CHUNKS):
                        evict(b, m, n, pss[n], aggb, on_act=(n % 2 == 0))

            def evict_rows(b, m, n, r0, rows, ps_sub, aggb, on_act):
                # eviction of a `rows`-row slice [r0, r0+rows) of chunk n
                ow = rows * W
                osb = outp.tile([128, ow], F16, name=f"osb{b}_{m}_{n}_{r0}",
                                tag="osb")
                ps_in = ps_sub.rearrange("p (y w) -> p y w", w=WP)[:, :, 1:W + 1]
                osb_out = osb[:, 0:ow].rearrange("p (y w) -> p y w", w=W)
                if on_act:
                    nc.scalar.activation(osb_out, ps_in, IDENT,
                                         bias=aggb[:, m:m + 1], scale=1.0)
                else:
                    nc.vector.tensor_scalar_add(osb_out, ps_in, aggb[:, m:m + 1])
                nc.sync.dma_start(
                    out=out[b, m * 128:(m + 1) * 128,
                            n * OW + r0 * W:n * OW + r0 * W + ow],
                    in_=osb[:, 0:ow],
                )

            def conv_mng(b, aggT, aggb, mids=None):
                """(m, n, g): chunk-serial, evictions spread (samples 1-3)."""
                mids = mids or {}
                for m in range(M_TILES):
                    for n in range(N_ROW_CHUNKS):
                        if (m, n) in mids:
                            mids[(m, n)]()
                        last = (TUNE["tail_split"] and b == B_LOC - 1
                                and m == M_TILES - 1 and n == N_ROW_CHUNKS - 1)
                        p0 = (ROWS_PER_CHUNK * n + 1) * WP
                        if not last:
                            ps = cpsp.tile([128, CW], F32,
                                           name=f"ps{b}_{m}_{n}", tag="ps")
                            for g in range(NG):
                                base = GUARD + p0 + TAP_DELTA[g]
                                nc.tensor.matmul(
                                    ps[:, 0:CW], lhsT=agg_flat(aggT, m, g, 1),
                                    rhs=xbufs[b][:, base:base + CW],
                                    start=(g == 0), stop=(g == NG - 1),
                                )
                            evict(b, m, n, ps, aggb, on_act=True)
                            continue
                        # final chunk: two 4-row sub-chunks so the tail's
                        # eviction + out-DMA pipeline is half as deep
                        HR = ROWS_PER_CHUNK // 2
                        CW2 = HR * WP
                        for h in range(2):
                            ph = p0 + h * HR * WP
                            ps = cpsp.tile([128, CW2], F32,
                                           name=f"ps{b}_{m}_{n}_{h}", tag="ps")
                            for g in range(NG):
                                base = GUARD + ph + TAP_DELTA[g]
                                nc.tensor.matmul(
                                    ps[:, 0:CW2], lhsT=agg_flat(aggT, m, g, 1),
                                    rhs=xbufs[b][:, base:base + CW2],
                                    start=(g == 0), stop=(g == NG - 1),
                                )
                            evict_rows(b, m, n, h * HR, HR, ps[:, 0:CW2],
                                       aggb, on_act=(h == 0))

            # ==== schedule ===============================================
            # PE stream: warmups / gap0 chunk matmuls / ones-reduce / conv...
            rp0 = route_head(0)
            warm(TUNE["warm_a"])
            half1 = [ci for ci in range(N_GAP_CHUNKS) if (ci + 1) * 128 <= XH]
            half2 = [ci for ci in range(N_GAP_CHUNKS) if ci not in half1]
            gap_mms(0, rp0, half1)
            warm(TUNE["warm_b"])
            gap_mms(0, rp0, half2)
            S0 = route_mid(0, rp0)        # DVE
            warm(TUNE["warm_c"])
            attn0 = route_tail(0, S0)     # PE ones-mm + ACT sigmoid
            warm(TUNE["warm_d"])

            # sample 0 streamed aggregation + (m,g,n) conv
            aggT0 = new_aggT(0)
            small_i = 0
            for m, groups in ((0, TUNE["s0_groups_m0"]),
                              (1, TUNE["s0_groups_m1"])):
                g0 = 0
                for ng in groups:
                    if ng == 1:
                        sl = TUNE["s0_small_list"]
                        if small_i < len(sl):
                            prods, pool_adds = sl[small_i]
                        else:
                            prods = TUNE["s0_prods_small"]
                            pool_adds = TUNE["s0_pool_pairs_small"]
                        small_i += 1
                    else:
                        prods = TUNE["s0_prods_big"]
                        pool_adds = TUNE["s0_pool_pairs_big"]
                    aggregate_group(0, attn0, aggT0, m, g0, ng,
                                    prods=prods, pool_adds=pool_adds)
                    g0 += ng

            rr = {}

            def mid_gap(b):
                def f():
                    rp = route_head(b)
                    gap_mms(b, rp, range(N_GAP_CHUNKS))
                    rr[f"rp{b}"] = rp
                return f

            def mid_agg_m0(b):
                def f():
                    S = route_mid(b, rr[f"rp{b}"])
                    attn = route_tail(b, S)
                    aggT = new_aggT(b)
                    aggregate_half(b, attn, aggT, 0)
                    rr[f"half{b}"] = (attn, aggT, agg_bias(b, attn))
                return f

            def mid_agg_m1(b):
                def f():
                    attn, aggT, aggb = rr[f"half{b}"]
                    aggregate_half(b, attn, aggT, 1)
                    rr[f"r{b}"] = (aggT, aggb)
                return f

            conv_mgn(0, aggT0, lambda: agg_bias(0, attn0),
                     mids={TUNE["mid1_gap"]: mid_gap(1),
                           TUNE["mid1_m0"]: mid_agg_m0(1),
                           TUNE["mid1_m1"]: mid_agg_m1(1)})

            conv_mng(1, rr["r1"][0], rr["r1"][1],
                     mids={(0, 2): mid_gap(2), (0, 4): mid_agg_m0(2),
                           (1, 1): mid_agg_m1(2)})

            conv_mng(2, rr["r2"][0], rr["r2"][1],
                     mids={(0, 2): mid_gap(3), (0, 4): mid_agg_m0(3),
                           (1, 1): mid_agg_m1(3)})
            conv_mng(3, rr["r3"][0], rr["r3"][1])

    _split_excess_waits(nc)
    return nc


def _get_nc():
    if "nc" not in _CACHE:
        _CACHE["nc"] = _build_bass()
    return _CACHE["nc"]


def _host_prep(x, fc_w, fc_b, weight, bias):
    # bank g-block-major: col = ((m*9 + g)*8 + k)*128 + co
    w6 = weight.astype(np.float32).reshape(K, M_TILES, 128, C_IN, KS * KS)
    # dims [k, m, co, c, g] -> [c, m, g, k, co]
    wT_host = np.ascontiguousarray(
        w6.transpose(3, 1, 4, 0, 2)
    ).reshape(C_IN, M_TILES * NG * BLK).astype(np.float16)

    # pre-padded, pre-cast xbuf payload: [B, C_IN, XBUF] fp16
    xp = np.zeros((B, C_IN, XBUF), dtype=np.float16)
    xpad = xp[:, :, GUARD:GUARD + NPAD].reshape(B, C_IN, H + 2, WP)
    xpad[:, :, 1:H + 1, 1:W + 1] = x.astype(np.float16)

    return xp, {
        "wT": wT_host,
        "fcwT": np.ascontiguousarray(fc_w.astype(np.float16).T),
        # fcb replicated per partition, pre-scaled by HW/128 so the
        # ones-matmul partition sum yields HW*(gap.fcw + fcb)
        "fcbb": np.ascontiguousarray(
            np.tile(fc_b.astype(np.float32).reshape(1, K) * (HW / 128.0),
                    (C_IN, 1))
        ),
        "biasT": np.ascontiguousarray(bias.astype(np.float32).T),
    }


def kernel(x, fc_w, fc_b, weight, bias):
    from concourse.bass_utils import run_bass_kernel_spmd

    x = np.asarray(x)
    fc_w, fc_b = np.asarray(fc_w), np.asarray(fc_b)
    weight, bias = np.asarray(weight), np.asarray(bias)

    nc = _get_nc()
    xp, shared = _host_prep(x, fc_w, fc_b, weight, bias)
    in_maps = [
        {"xs": xp[c * B_LOC:(c + 1) * B_LOC], **shared} for c in range(N_CORES)
    ]
    res = run_bass_kernel_spmd(nc, in_maps, core_ids=list(range(N_CORES)))
    _CACHE["last_res"] = res
    full = np.concatenate([r["out"] for r in res.results], axis=0)
    return full.reshape(B, C_OUT, H, W).astype(np.float32)


if __name__ == "__main__":
    rng = np.random.default_rng(0)
    x = rng.standard_normal((B, C_IN, H, W), dtype=np.float32)
    fc_w = rng.standard_normal((K, C_IN), dtype=np.float32) * 0.05
    fc_b = rng.standard_normal((K,), dtype=np.float32) * 0.05
    weight = rng.standard_normal((K, C_OUT, C_IN, KS, KS), dtype=np.float32) * 0.05
    bias = rng.standard_normal((K, C_OUT), dtype=np.float32) * 0.05
    out = kernel(x, fc_w, fc_b, weight, bias)
    print(out.shape, out.dtype, np.abs(out).mean())
